# revision 13
# baseline (speedup 1.0000x reference)
"""Bidirectional LSTM (B=16, S=2048, D=H=512) on 8 NeuronCores.

Sharding: direction x batch. Cores 0-3 run the forward scan on batch
shards of 4 examples; cores 4-7 run the backward scan (implemented as a
forward scan over time-reversed input, flipped back on the host). All 8
cores run the same SPMD program.

Per-core program:
  Phase 1: xW = x @ W + b as one big GEMM (x transposed on the PE,
           fp32r matmuls, N-chunks of 512), spilled to a DRAM tile in
           [s, b, 4H] layout (s-major token tiles so each scan-step read
           hits exactly one producer DMA).
  Phase 2: 2048-step LSTM scan. Per step: z = xW_t + h_{t-1} @ U via 16
           fp32r matmuls (h^T stationary [128,4], U moving [128,512]),
           gate math on ACT/DVE, h transposed back to [H,B] layout on
           the PE for the next step's stationary operand.

Hardware constraint baked into the structure: a Matmult instruction can
carry at most ONE sync-wait, so weight tensors are loaded by single DMA
instructions and "joined" onto the PE timeline by tiny dummy matmuls
before use.
"""

import numpy as np

try:
    import concourse.bass as bass
except ImportError:  # pragma: no cover
    import sys

    sys.path.insert(0, "/opt/trn_rl_repo")
    import concourse.bass as bass

import concourse.bacc as bacc
import concourse.mybir as mybir
import concourse.tile as tile
from concourse.tile import add_dep_helper
from concourse.bass_utils import run_bass_kernel_spmd

AF = mybir.ActivationFunctionType

B, S, D, H = 16, 2048, 512, 512
G4 = 4 * H          # gate width: 2048 (order i, f, g, o)
P = 128             # SBUF partitions
NK = D // P         # 4 contraction chunks
NH = G4 // 512      # 4 N-chunks of 512
N_CORES = 8
N_SHARDS = 4        # batch shards per direction
BC = B // N_SHARDS  # 4 examples per core
SPT = P // BC       # 32 s-steps per token tile
OUT_CHUNK = 8       # scan steps buffered per seq DMA

_PROGRAM_CACHE: dict[int, "bass.Bass"] = {}
TRACE = False          # set True to capture an NTFF profile on the next run
LAST_EXEC_NS = None    # HW exec time of the most recent traced run
LAST_RESULT = None     # full BassKernelResults of the most recent run


def _build_program(s_len: int) -> "bass.Bass":
    f32 = mybir.dt.float32
    f32r = mybir.dt.float32r
    nc = bacc.Bacc(
        "TRN2", target_bir_lowering=False, debug=False, num_devices=N_CORES
    )

    xs = nc.declare_dram_parameter("xs", [BC, s_len, D], f32, isOutput=False)
    Wt = nc.declare_dram_parameter("Wt", [D, G4], f32, isOutput=False)
    Ut = nc.declare_dram_parameter("Ut", [H, G4], f32, isOutput=False)
    bv = nc.declare_dram_parameter("bv", [1, G4], f32, isOutput=False)
    idn = nc.declare_dram_parameter("idn", [P, P], f32, isOutput=False)
    seq = nc.declare_dram_parameter("seq", [BC, s_len, H], f32, isOutput=True)
    h_last = nc.declare_dram_parameter("h_last", [BC, H], f32, isOutput=True)
    c_last = nc.declare_dram_parameter("c_last", [BC, H], f32, isOutput=True)

    n_tt = s_len // SPT  # token tiles (128 tokens each, s-major)

    with tile.TileContext(nc) as tc:
        with (
            tc.tile_pool(name="dram", bufs=1, space="DRAM") as dramp,
            tc.tile_pool(name="const", bufs=1) as constp,
        ):
            xw = dramp.tile([BC, s_len, G4], f32)

            idn_sb = constp.tile([P, P], f32)
            nc.sync.dma_start(idn_sb[:], idn[:])

            # ---------------- Phase 1: xW = x @ W + b ----------------
            with (
                tc.tile_pool(name="p1w", bufs=1) as wpool,
                tc.tile_pool(name="p1xr", bufs=3) as xrpool,
                tc.tile_pool(name="p1x", bufs=3) as xpool,
                tc.tile_pool(name="p1o", bufs=3) as opool,
                tc.tile_pool(name="p1ps", bufs=1, space="PSUM") as pspool,
                tc.tile_pool(name="p1tp", bufs=4, space="PSUM") as txpool,
            ):
                W_sb = wpool.tile([P, NK * G4], f32r)
                nc.gpsimd.dma_start(
                    W_sb[:].rearrange("p (k g) -> p k g", k=NK),
                    Wt.rearrange("(k p) g -> p k g", p=P),
                )
                # join idn's and W's DMAs onto the PE timeline once, so the
                # per-tile matmuls below each need only a single sync-wait
                dj = txpool.tile([P, P], f32, tag="tpx")
                nc.tensor.matmul(dj[0:1, 0:1], idn_sb[0:1, 0:1],
                                 idn_sb[0:1, 0:1], start=True, stop=True)
                dj2 = txpool.tile([P, P], f32, tag="tpx")
                nc.tensor.matmul(dj2[0:1, 0:1], W_sb[0:1, 0:1].bitcast(f32),
                                 W_sb[0:1, 0:1].bitcast(f32), start=True, stop=True)
                btile = wpool.tile([P, G4], f32)
                nc.sync.dma_start(btile[:], bv[0:1, :].broadcast_to((P, G4)))

                xw_writers = []
                for tt in range(n_tt):
                    s0 = tt * SPT
                    xrow = xrpool.tile([P, D], f32, tag="xrow")
                    for b4 in range(BC):
                        nc.sync.dma_start(
                            xrow[b4 * SPT : (b4 + 1) * SPT, :],
                            xs[b4, s0 : s0 + SPT, :],
                        )
                    xT = xpool.tile([P, NK * P], f32r, tag="xT")
                    for kc in range(NK):
                        tpx = txpool.tile([P, P], f32, tag="tpx")
                        nc.tensor.transpose(
                            tpx[:], xrow[:, kc * P : (kc + 1) * P], idn_sb[:]
                        )
                        nc.vector.tensor_copy(xT[:, kc * P : (kc + 1) * P], tpx[:])
                    ps = pspool.tile([P, G4], f32, tag="ps")
                    for nn in range(NH):
                        nsl = slice(nn * 512, (nn + 1) * 512)
                        for kc in range(NK):
                            nc.tensor.matmul(
                                ps[:, nsl],
                                xT[:, kc * P : (kc + 1) * P],
                                W_sb[:, kc * G4 + nn * 512 : kc * G4 + (nn + 1) * 512],
                                start=(kc == 0),
                                stop=(kc == NK - 1),
                            )
                    xo = opool.tile([P, G4], f32, tag="xo")
                    for nn in range(NH):
                        nsl = slice(nn * 512, (nn + 1) * 512)
                        nc.vector.tensor_add(xo[:, nsl], ps[:, nsl], btile[:, nsl])
                    xw_writers.append([
                        nc.sync.dma_start(
                            xw[b4, s0 : s0 + SPT, :],
                            xo[b4 * SPT : (b4 + 1) * SPT, :],
                        )
                        for b4 in range(BC)
                    ])

            # ---------------- Phase 2: LSTM scan ----------------
            with (
                tc.tile_pool(name="upool", bufs=1) as upool,
                tc.tile_pool(name="state", bufs=2) as statep,
                tc.tile_pool(name="xwt", bufs=8) as xwp,
                tc.tile_pool(name="gates", bufs=2) as gp,
                tc.tile_pool(name="hout", bufs=2) as hop,
                tc.tile_pool(name="zps", bufs=6, space="PSUM") as zpool,
                tc.tile_pool(name="tps", bufs=1, space="PSUM") as tpool,
            ):
                U_sb = upool.tile([P, NK * G4], f32r)
                nc.gpsimd.dma_start(
                    U_sb[:].rearrange("p (k g) -> p k g", k=NK),
                    Ut.rearrange("(k p) g -> p k g", p=P),
                )
                # join U's DMA onto the PE timeline
                dj3 = tpool.tile([P, NK * BC], f32, tag="tp")
                nc.tensor.matmul(dj3[0:1, 0:1], U_sb[0:1, 0:1].bitcast(f32),
                                 U_sb[0:1, 0:1].bitcast(f32), start=True, stop=True)

                hT = statep.tile([P, NK * BC], f32r, tag="hT")
                zst = statep.tile([P, NK * BC], f32, tag="zst")
                nc.gpsimd.memset(zst[:], 0.0)
                nc.vector.tensor_copy(hT[:], zst[:])
                c_prev = statep.tile([BC, H], f32, tag="c")
                nc.gpsimd.memset(c_prev[:], 0.0)

                goff = {"i": 0, "f": 512, "g": 1024, "o": 1536}
                hout = None
                for t in range(s_len):
                    if t % OUT_CHUNK == 0:
                        hout = hop.tile([BC, OUT_CHUNK * H], f32, tag="hout")
                    xwt = xwp.tile([BC, G4], f32, tag="xwt")
                    xwt_dma = nc.sync.dma_start(xwt[:], xw[0:BC, t, :])
                    # DRAM-tile RAW deps are not tracked by Tile; wire the
                    # phase-1 writer -> scan reader edges explicitly.
                    for winst in xw_writers[t // SPT]:
                        add_dep_helper(xwt_dma.ins, winst.ins,
                                       sync=True, reason="xw RAW")

                    # z = h_{t-1} @ U, one PSUM tile per gate, in the order
                    # the gate pipeline consumes them.
                    z = {}
                    for gname in ("g", "i", "f", "o"):
                        off = goff[gname]
                        zt = zpool.tile([BC, 512], f32, tag="z")
                        for kc in range(NK):
                            nc.tensor.matmul(
                                zt[:],
                                hT[:, kc * BC : (kc + 1) * BC],
                                U_sb[:, kc * G4 + off : kc * G4 + off + 512],
                                start=(kc == 0),
                                stop=(kc == NK - 1),
                            )
                        z[gname] = zt

                    def gate(gname, func, tag):
                        zs = gp.tile([BC, H], f32, tag="z" + tag)
                        nc.vector.tensor_add(
                            zs[:], z[gname][:],
                            xwt[:, goff[gname] : goff[gname] + H],
                        )
                        out = gp.tile([BC, H], f32, tag="a" + tag)
                        nc.scalar.activation(out[:], zs[:], func)
                        return out

                    tg = gate("g", AF.Tanh, "g")
                    si = gate("i", AF.Sigmoid, "i")
                    ig = gp.tile([BC, H], f32, tag="ig")
                    nc.vector.tensor_mul(ig[:], si[:], tg[:])
                    sf = gate("f", AF.Sigmoid, "f")
                    fc = gp.tile([BC, H], f32, tag="fc")
                    nc.vector.tensor_mul(fc[:], sf[:], c_prev[:])
                    c_new = statep.tile([BC, H], f32, tag="c")
                    nc.vector.tensor_add(c_new[:], fc[:], ig[:])
                    tcell = gp.tile([BC, H], f32, tag="tc")
                    nc.scalar.activation(tcell[:], c_new[:], AF.Tanh)
                    so = gate("o", AF.Sigmoid, "o")

                    ho = t % OUT_CHUNK
                    hsl = hout[:, ho * H : (ho + 1) * H]
                    nc.vector.tensor_mul(hsl, so[:], tcell[:])

                    if t < s_len - 1:
                        # h -> h^T for the next step's stationary operand
                        tp = tpool.tile([P, NK * BC], f32, tag="tp")
                        for kc in range(NK):
                            nc.tensor.transpose(
                                tp[:, kc * BC : (kc + 1) * BC],
                                hout[0:BC, ho * H + kc * P : ho * H + (kc + 1) * P],
                                idn_sb[0:BC, 0:BC],
                            )
                        hT = statep.tile([P, NK * BC], f32r, tag="hT")
                        nc.vector.tensor_copy(hT[:], tp[:])

                    c_prev = c_new
                    if (t + 1) % OUT_CHUNK == 0:
                        t0 = t + 1 - OUT_CHUNK
                        nc.sync.dma_start(
                            seq[0:BC, t0 : t0 + OUT_CHUNK, :]
                            .rearrange("b t h -> b (t h)"),
                            hout[:],
                        )
                    if t == s_len - 1:
                        nc.sync.dma_start(h_last[:], hsl)
                        nc.sync.dma_start(c_last[:], c_new[:])

    nc.compile()
    return nc


def _get_program(s_len: int) -> "bass.Bass":
    if s_len not in _PROGRAM_CACHE:
        _PROGRAM_CACHE[s_len] = _build_program(s_len)
    return _PROGRAM_CACHE[s_len]


def _run(x, W_fw, U_fw, b_fw, W_bw, U_bw, b_bw):
    b_total, s_len, _ = x.shape
    nc = _get_program(s_len)
    ident = np.eye(P, dtype=np.float32)

    in_maps = []
    for core in range(N_CORES):
        shard = core % N_SHARDS
        xsl = x[shard * BC : (shard + 1) * BC]
        if core < N_SHARDS:  # forward cores
            Wc, Uc, bc = W_fw, U_fw, b_fw
        else:  # backward cores: forward scan over time-reversed input
            xsl = xsl[:, ::-1, :]
            Wc, Uc, bc = W_bw, U_bw, b_bw
        in_maps.append(
            {
                "xs": np.ascontiguousarray(xsl, dtype=np.float32),
                "Wt": np.ascontiguousarray(Wc, dtype=np.float32),
                "Ut": np.ascontiguousarray(Uc, dtype=np.float32),
                "bv": np.ascontiguousarray(bc, dtype=np.float32).reshape(1, G4),
                "idn": ident,
            }
        )

    global LAST_EXEC_NS, LAST_RESULT
    br = run_bass_kernel_spmd(
        nc, in_maps, list(range(N_CORES)), trace=TRACE
    )
    LAST_RESULT = br
    if br.exec_time_ns is not None:
        LAST_EXEC_NS = br.exec_time_ns
    res = br.results

    out = np.empty((b_total, s_len, 2 * H), dtype=np.float32)
    h_fw = np.empty((b_total, H), dtype=np.float32)
    c_fw = np.empty((b_total, H), dtype=np.float32)
    h_bw = np.empty((b_total, H), dtype=np.float32)
    c_bw = np.empty((b_total, H), dtype=np.float32)
    for core in range(N_CORES):
        shard = core % N_SHARDS
        rows = slice(shard * BC, (shard + 1) * BC)
        r = res[core]
        if core < N_SHARDS:
            out[rows, :, :H] = r["seq"]
            h_fw[rows] = r["h_last"]
            c_fw[rows] = r["c_last"]
        else:
            out[rows, :, H:] = r["seq"][:, ::-1, :]
            h_bw[rows] = r["h_last"]
            c_bw[rows] = r["c_last"]
    return out, h_fw, c_fw, h_bw, c_bw


def kernel(x, W_fw, U_fw, b_fw, W_bw, U_bw, b_bw):
    return _run(
        np.asarray(x), np.asarray(W_fw), np.asarray(U_fw), np.asarray(b_fw),
        np.asarray(W_bw), np.asarray(U_bw), np.asarray(b_bw),
    )
